# revision 17
# baseline (speedup 1.0000x reference)
"""Trainium2 Bass kernel for nn_Attention_16071767621814.

MobileViT-style attention block: 3x (depthwise3x3 conv + BN + 1x1 pointwise)
for q/k/v, 8-head attention (Lq=1024, Lkv=256, d=64), head-mixing reshape,
1x1 output projection.

Sharding: pure data-parallel over batch (16 batches / 8 cores = 2 per core),
zero collectives.

Per-core design (all layouts [feature-on-partition, token-on-free]):
- x arrives bf16 from host (halves input DMA), staged contiguously then
  pad-copied on Vector (4x copy mode).
- BN folded host-side: scale into dw weights, bias via pw const row.
- batch-0 depthwise q conv as 9 shifted diagonal matmuls on the (otherwise
  idle) PE during the prologue; batch-1 q conv as Vector/GpSimd taps.
- xqb holds the q dw-conv output in NATURAL lq order; the head-mix token
  reorder (i = m*128 + j for lq = j*8 + m) is folded into the q pointwise
  matmul's rhs access pattern (strided rhs stream costs the PE nothing).
- k/v stride-2 convs read the padded tile directly with stride-2 views.
- S^T = k^T q on PE, exp on ScalarE, denominator via ones-lhsT matmuls
  (emitted BEFORE av so the reciprocal overlaps av matmuls).
- normalize fused into the head-mix refold: 4 strided tensor_tensor
  multiplies per head-pair writing F in place, split Vector/GpSimd.
- o_proj as 16 matmuls of N=512,K=128 per batch from F.
- PE warm-up dummy matmuls during the DMA prologue (avoids the ~1.2GHz
  cold p-state on the first ~4us of real matmuls).
"""

import numpy as np
import ml_dtypes
BF16NP = ml_dtypes.bfloat16

from concourse import bass, bacc, tile, mybir
from concourse.bass_utils import run_bass_kernel_spmd

F32 = mybir.dt.float32
BF16 = mybir.dt.bfloat16
AF = mybir.ActivationFunctionType
OP = mybir.AluOpType

NCORES = 8
B, C, S = 16, 192, 32
BPC = B // NCORES          # 2 batches per core
Lq = S * S                 # 1024
Sk = S // 2                # 16
Lkv = Sk * Sk              # 256
HEADS, HID, INNER = 8, 64, 512
EPS = 1e-5
PS = S + 2                 # padded spatial 34
NWARM = 18                 # PE warm-up dummies

_NC = None
LAST_RESULT = None


def _build():
    nc = bacc.Bacc("TRN2", target_bir_lowering=False, debug=False,
                   num_devices=NCORES)

    x_ext = nc.declare_dram_parameter("x", [BPC, C, Lq], BF16, isOutput=False)
    pw_ext = {}
    dw_ext = {}
    for p in ("q", "k", "v"):
        pw_ext[p] = nc.declare_dram_parameter(p + "pwT", [C + 1, INNER], BF16,
                                              isOutput=False)
        dw_ext[p] = nc.declare_dram_parameter(p + "dw", [C, 9], F32,
                                              isOutput=False)
    owt4_ext = nc.declare_dram_parameter("owt4", [128, 4 * C], BF16,
                                         isOutput=False)
    qdA_ext = nc.declare_dram_parameter("qdiagA", [128, 9 * 128], BF16,
                                        isOutput=False)
    qdB_ext = nc.declare_dram_parameter("qdiagB", [64, 9 * 64], BF16,
                                        isOutput=False)
    ones_ext = nc.declare_dram_parameter("ones128x64", [128, HID], BF16,
                                         isOutput=False)
    onesq_ext = nc.declare_dram_parameter("onesq", [1, BPC, Lq], BF16,
                                          isOutput=False)
    oneskv_ext = nc.declare_dram_parameter("oneskv", [1, BPC, Lkv], BF16,
                                           isOutput=False)
    ob_ext = nc.declare_dram_parameter("ob", [C, 1], F32, isOutput=False)
    out_ext = nc.declare_dram_parameter("out", [BPC, C, Lq], F32, isOutput=True)

    TAPS = [(dy, dx) for dy in range(3) for dx in range(3)]

    from contextlib import ExitStack
    with tile.TileContext(nc) as tc, ExitStack() as ctx:
        const = ctx.enter_context(tc.tile_pool(name="const", bufs=1))
        xpool = ctx.enter_context(tc.tile_pool(name="xpool", bufs=1))
        wpool = ctx.enter_context(tc.tile_pool(name="wpool", bufs=2))
        psw = ctx.enter_context(tc.tile_pool(name="psw", bufs=4, space="PSUM"))

        # ---- PE warm-up: dummy matmuls on memset tiles, no ext deps ----
        warmw = const.tile([128, 128], BF16, name="warmw")
        warmr = const.tile([128, 512], BF16, name="warmr")
        nc.vector.memset(warmw[:], 0.0)
        nc.vector.memset(warmr[:], 0.0)
        for _ in range(NWARM):
            wp = psw.tile([128, 512], F32, name="work")
            nc.tensor.matmul(wp[:], warmw[:], warmr[:], start=True, stop=True)

        # ---- x staging DMA (contiguous, fast); batch 0 + diag weights
        # first so the prologue diag-conv isn't starved ----
        xsA = xpool.tile([128, BPC, Lq], BF16, name="xsA")
        xsB = xpool.tile([C - 128, BPC, Lq], BF16, name="xsB")
        nc.sync.dma_start(out=xsA[:, 0, :], in_=x_ext[0, 0:128, :])
        nc.sync.dma_start(out=xsB[:, 0, :], in_=x_ext[0, 128:C, :])
        qdA = const.tile([128, 9, 128], BF16, name="qdA")
        nc.sync.dma_start(out=qdA[:],
                          in_=qdA_ext[:].rearrange("p (t c) -> p t c", t=9))
        qdB = const.tile([64, 9, 64], BF16, name="qdB")
        nc.sync.dma_start(out=qdB[:],
                          in_=qdB_ext[:].rearrange("p (t c) -> p t c", t=9))

        # padded bf16 input tiles (zero borders only; interior overwritten)
        xpA = xpool.tile([128, BPC, PS, PS], BF16, name="xpA")
        xpB = xpool.tile([C - 128, BPC, PS, PS], BF16, name="xpB")
        for xp_t in (xpA, xpB):
            nc.vector.memset(xp_t[:, :, 0:1, :], 0.0)
            nc.vector.memset(xp_t[:, :, S + 1:S + 2, :], 0.0)
            nc.vector.memset(xp_t[:, :, :, 0:1], 0.0)
            nc.vector.memset(xp_t[:, :, :, S + 1:S + 2], 0.0)

        def emit_convert(bi):
            # bf16 pad-copy on Vector (4x copy mode: packed bf16, SBUF only)
            nc.vector.tensor_copy(
                xpA[:, bi, 1:S + 1, 1:S + 1],
                xsA[:, bi].rearrange("p (h w) -> p h w", h=S))
            nc.vector.tensor_copy(
                xpB[:, bi, 1:S + 1, 1:S + 1],
                xsB[:, bi].rearrange("p (h w) -> p h w", h=S))

        emit_convert(0)

        # ---- weights to SBUF ----
        pwA, pwB, dwA, dwB = {}, {}, {}, {}
        for p in ("q", "k", "v"):
            pwA[p] = const.tile([128, INNER], BF16, name=f"pwA{p}")
            pwB[p] = const.tile([C + 1 - 128, INNER], BF16, name=f"pwB{p}")
            nc.sync.dma_start(out=pwA[p][:], in_=pw_ext[p][0:128, :])
            nc.sync.dma_start(out=pwB[p][:], in_=pw_ext[p][128:C + 1, :])
            dwA[p] = const.tile([128, 9], F32, name=f"dwA{p}")
            dwB[p] = const.tile([C - 128, 9], F32, name=f"dwB{p}")
            nc.sync.dma_start(out=dwA[p][:], in_=dw_ext[p][0:128, :])
            nc.sync.dma_start(out=dwB[p][:], in_=dw_ext[p][128:C, :])
        owt4 = const.tile([128, 4, C], BF16, name="owt4")
        nc.sync.dma_start(out=owt4[:],
                          in_=owt4_ext[:].rearrange("p (k c) -> p k c", k=4))
        # batch-1 staging after the weight DMAs (not prologue-critical)
        nc.sync.dma_start(out=xsA[:, 1, :], in_=x_ext[1, 0:128, :])
        nc.sync.dma_start(out=xsB[:, 1, :], in_=x_ext[1, 128:C, :])
        o_b0 = const.tile([128, 1], F32, name="ob0")
        o_b1 = const.tile([C - 128, 1], F32, name="ob1")
        nc.sync.dma_start(out=o_b0[:], in_=ob_ext[0:128, :])
        nc.sync.dma_start(out=o_b1[:], in_=ob_ext[128:C, :])
        ones64 = const.tile([128, HID], BF16, name="ones64")
        nc.sync.dma_start(out=ones64[:], in_=ones_ext[:])

        # ---- dw conv output tiles (bf16), ones row at partition 64 of B ----
        xqb = {"A": xpool.tile([128, BPC, Lq], BF16, name="xqbA"),
               "B": xpool.tile([65, BPC, Lq], BF16, name="xqbB")}
        xkb = {"A": xpool.tile([128, BPC, Lkv], BF16, name="xkbA"),
               "B": xpool.tile([65, BPC, Lkv], BF16, name="xkbB")}
        xvb = {"A": xpool.tile([128, BPC, Lkv], BF16, name="xvbA"),
               "B": xpool.tile([65, BPC, Lkv], BF16, name="xvbB")}
        nc.sync.dma_start(out=xqb["B"][64:65, :, :], in_=onesq_ext[:])
        nc.sync.dma_start(out=xkb["B"][64:65, :, :], in_=oneskv_ext[:])
        nc.sync.dma_start(out=xvb["B"][64:65, :, :], in_=oneskv_ext[:])

        # batch-1 q dw-conv accumulators (natural lq order) + tap scratch
        aq = {"A": xpool.tile([128, Lq], BF16, name="aqA"),
              "B": xpool.tile([64, Lq], BF16, name="aqB")}
        tmpq = {"A": xpool.tile([128, Lq], BF16, name="tmpqA"),
                "B": xpool.tile([64, Lq], BF16, name="tmpqB")}

        def emit_dw_q_taps(bi, tile_sel, lo=0, hi=9):
            # taps for one tile half into aq; tensor_scalar (4x mode) +
            # tensor_tensor add (2x mode) beats 1x scalar_tensor_tensor
            srcT, dwt = ((xpA, dwA["q"]) if tile_sel == "A"
                         else (xpB, dwB["q"]))
            np_ = min(srcT.shape[0], 128)
            outap = aq[tile_sel][0:np_].rearrange("p (h w) -> p h w", h=S)
            tmpap = tmpq[tile_sel][0:np_].rearrange("p (h w) -> p h w", h=S)
            for t in range(lo, hi):
                dy, dx = TAPS[t]
                inap = srcT[0:np_, bi, dy:dy + S, dx:dx + S]
                if t == 0:
                    nc.vector.tensor_scalar(outap, inap, dwt[:, 0:1],
                                            None, OP.mult)
                else:
                    nc.vector.tensor_scalar(tmpap, inap, dwt[:, t:t + 1],
                                            None, OP.mult)
                    nc.vector.tensor_tensor(outap, outap, tmpap, OP.add)

        def emit_dw_q_b1_reorder():
            # aq (natural) -> xqb (head-mix order i = m*128 + j), on Scalar
            # to keep the Vector queue free for the attention chains
            for tl in ("A", "B"):
                np_ = 128 if tl == "A" else 64
                src_ap = aq[tl][0:np_].rearrange("p (j m) -> p m j", m=8)
                dst_ap = xqb[tl][0:np_, 1].rearrange("p (m j) -> p m j", m=8)
                nc.scalar.copy(dst_ap, src_ap)

        def emit_dw_q_pe(bi, tile_sel):
            # depthwise q conv as 9 accumulating diagonal matmuls on PE
            qd, xp_t, np_ = ((qdA, xpA, 128) if tile_sel == "A"
                             else (qdB, xpB, 64))
            qcp = psw.tile([np_, Lq], F32, name="work")
            for n in range(2):
                ns = slice(n * 512, (n + 1) * 512)
                for t, (dy, dx) in enumerate(TAPS):
                    rhs = xp_t[0:np_, bi, dy + 16 * n:dy + 16 * n + 16,
                               dx:dx + S]
                    nc.tensor.matmul(qcp[:, ns], qd[:, t, :], rhs,
                                     start=(t == 0), stop=(t == 8))
            return qcp

        def emit_dw_q_cast(bi, src_tiles):
            # psum -> sbuf bf16 cast fused with the head-mix reorder
            # (strided src costs the copy engine nothing extra)
            for tl, srcT in src_tiles.items():
                np_ = 128 if tl == "A" else 64
                src_ap = srcT[0:np_].rearrange("p (j m) -> p m j", m=8)
                dst_ap = xqb[tl][0:np_, bi].rearrange("p (m j) -> p m j", m=8)
                nc.scalar.copy(dst_ap, src_ap)

        def emit_dw_kv(bi, p, tiles=("A", "B"), eng=None):
            # stride-2 taps reading the padded tile directly
            eng = eng or nc.vector
            dst = xkb if p == "k" else xvb
            for ti, (xpT, dwt) in enumerate(((xpA, dwA[p]), (xpB, dwB[p]))):
                if ("A", "B")[ti] not in tiles:
                    continue
                np_ = min(xpT.shape[0], 128)
                d = dst["A"] if ti == 0 else dst["B"]
                outap = d[0:np_, bi].rearrange("p (i j) -> p i j", i=Sk)
                for t, (dy, dx) in enumerate(TAPS):
                    inap = xpT[0:np_, bi, dy:dy + 2 * Sk:2, dx:dx + 2 * Sk:2]
                    if t == 0:
                        eng.tensor_scalar(outap, inap, dwt[:, 0:1],
                                          None, OP.mult)
                    else:
                        eng.scalar_tensor_tensor(
                            outap, inap, dwt[:, t:t + 1], outap,
                            OP.mult, OP.add)

        # ---- per-batch compute; q mms first so PE isn't gated on k/v taps --
        def pw_block(bi):
            q_sb, k_sb, vT_sb = [], [], []
            for hp in range(4):
                hs = slice(hp * 128, (hp + 1) * 128)
                qp = psw.tile([128, Lq], F32, name="work")
                for n in range(2):
                    ns = slice(n * 512, (n + 1) * 512)
                    nc.tensor.matmul(qp[:, ns], pwA["q"][:, hs],
                                     xqb["A"][:, bi, ns],
                                     start=True, stop=False)
                    nc.tensor.matmul(qp[:, ns], pwB["q"][:, hs],
                                     xqb["B"][:, bi, ns],
                                     start=False, stop=True)
                qs = wpool.tile([128, Lq], BF16, name=f"qsb{hp}", bufs=2)
                nc.scalar.copy(qs[:], qp[:])
                q_sb.append(qs)
            for hp in range(4):
                hs = slice(hp * 128, (hp + 1) * 128)
                kp = psw.tile([128, Lkv], F32, name="work")
                nc.tensor.matmul(kp[:], pwA["k"][:, hs],
                                 xkb["A"][:, bi, :],
                                 start=True, stop=False)
                nc.tensor.matmul(kp[:], pwB["k"][:, hs],
                                 xkb["B"][:, bi, :],
                                 start=False, stop=True)
                ks = wpool.tile([128, Lkv], BF16, name=f"ksb{hp}", bufs=2)
                nc.scalar.copy(ks[:], kp[:])
                k_sb.append(ks)
            for kc in range(2):
                vp = psw.tile([128, INNER], F32, name="work")
                kvs = slice(kc * 128, (kc + 1) * 128)
                nc.tensor.matmul(vp[:], xvb["A"][:, bi, kvs],
                                 pwA["v"][:], start=True, stop=False)
                nc.tensor.matmul(vp[:], xvb["B"][:, bi, kvs],
                                 pwB["v"][:], start=False, stop=True)
                vs = wpool.tile([128, INNER], BF16, name=f"vtsb{kc}", bufs=2)
                nc.scalar.copy(vs[:], vp[:])
                vT_sb.append(vs)
            return q_sb, k_sb, vT_sb

        Fs = {}

        def attn_st(bi, hp, q_sb, k_sb, vT_sb):
            if hp == 0:
                Fs[bi] = wpool.tile([128, 4, Lq], BF16, name="Ftile", bufs=2)
            h0, h1 = 2 * hp, 2 * hp + 1
            qs_t, ks_t = q_sb[hp], k_sb[hp]
            expAs = {}
            for idx, h in enumerate((h0, h1)):
                R = slice(idx * HID, (idx + 1) * HID)
                for kc in range(2):
                    st = psw.tile([128, Lq], F32, name="work")
                    kvs = slice(kc * 128, (kc + 1) * 128)
                    for n in range(2):
                        ns = slice(n * 512, (n + 1) * 512)
                        nc.tensor.matmul(st[:, ns],
                                         ks_t[R, kvs],
                                         qs_t[R, ns],
                                         start=True, stop=True)
                    ex = wpool.tile([128, Lq], BF16, name="expA", bufs=8)
                    nc.scalar.activation(ex[:], st[:], AF.Exp,
                                         scale=1.0 / (HID ** 0.5))
                    expAs[(h, kc)] = ex
            return expAs

        deferred_refolds = []

        def attn_av(bi, hp, expAs, q_sb, k_sb, vT_sb):
            F = Fs[bi]
            h0, h1 = 2 * hp, 2 * hp + 1
            # dnp allocated first so pw/o_proj psum slots recycle onto the
            # recip-freed dnp rather than the late-freed avp
            dnp = psw.tile([128, Lq], F32, name="work")
            avp = psw.tile([128, Lq], F32, name="work")
            # denominator matmuls first: the reciprocal overlaps av matmuls
            for idx, h in enumerate((h0, h1)):
                tp = (0, 0) if idx == 0 else (0, 64)
                rows = slice(idx * HID, (idx + 1) * HID)
                for kc in range(2):
                    for n in range(2):
                        ns = slice(n * 512, (n + 1) * 512)
                        nc.tensor.matmul(dnp[rows, ns], ones64[:, :],
                                         expAs[(h, kc)][:, ns],
                                         start=(kc == 0), stop=(kc == 1),
                                         tile_position=tp)
            rc = wpool.tile([128, Lq], F32, name="recip", bufs=2)
            nc.vector.reciprocal_approx_fast(rc[:], dnp[:])
            for idx, h in enumerate((h0, h1)):
                tp = (0, 0) if idx == 0 else (0, 64)
                rows = slice(idx * HID, (idx + 1) * HID)
                hs = slice(h * HID, (h + 1) * HID)
                for kc in range(2):
                    for n in range(2):
                        ns = slice(n * 512, (n + 1) * 512)
                        nc.tensor.matmul(avp[rows, ns],
                                         vT_sb[kc][:, hs],
                                         expAs[(h, kc)][:, ns],
                                         start=(kc == 0), stop=(kc == 1),
                                         tile_position=tp)
            # normalize (V) then head-mix refold; par0 on Vector now,
            # par1 stashed for deferred Scalar emission so exp streams on
            # Scalar aren't queued behind refold copies
            avbf = wpool.tile([128, Lq], BF16, name="avbf", bufs=4)
            nc.vector.tensor_tensor(avbf[:], avp[:], rc[:], OP.mult)
            for idx, h in enumerate((h0, h1)):
                rows = slice(idx * HID, (idx + 1) * HID)
                src4 = avbf[rows, :].rearrange("p (a b j) -> p a b j",
                                               a=4, b=2)
                dst = F[0:HID, :, h * 128:(h + 1) * 128]
                nc.vector.tensor_copy(dst, src4[:, :, 0, :])
                deferred_refolds.append(
                    (F[HID:2 * HID, :, h * 128:(h + 1) * 128],
                     src4[:, :, 1, :]))

        def flush_refolds():
            while deferred_refolds:
                dst, src = deferred_refolds.pop(0)
                nc.scalar.copy(dst, src)

        def o_proj_half(bi, half, split_tail=False):
            cs = slice(half * 512, (half + 1) * 512)
            P0 = psw.tile([128, 512], F32, name="work")
            P1 = psw.tile([C - 128, 512], F32, name="work")
            for kk in range(4):
                nc.tensor.matmul(P0[:], owt4[:, kk, 0:128],
                                 Fs[bi][:, kk, cs],
                                 start=(kk == 0), stop=(kk == 3))
                nc.tensor.matmul(P1[:], owt4[:, kk, 128:C],
                                 Fs[bi][:, kk, cs],
                                 start=(kk == 0), stop=(kk == 3))
            os0 = wpool.tile([128, 512], F32, name="os0", bufs=2)
            os1 = wpool.tile([C - 128, 512], F32, name="os1", bufs=2)
            if split_tail:
                for c0 in range(0, 512, 256):
                    cc = slice(c0, c0 + 256)
                    nc.scalar.activation(os0[:, cc], P0[:, cc], AF.Identity,
                                         bias=o_b0[:])
                    nc.vector.tensor_scalar(os1[:, cc], P1[:, cc], o_b1[:],
                                            None, OP.add)
                    nc.sync.dma_start(out=out_ext[bi, 0:128, half * 512 + c0:
                                                  half * 512 + c0 + 256],
                                      in_=os0[:, cc])
                    nc.sync.dma_start(out=out_ext[bi, 128:C, half * 512 + c0:
                                                  half * 512 + c0 + 256],
                                      in_=os1[:, cc])
            else:
                nc.scalar.activation(os0[:], P0[:], AF.Identity, bias=o_b0[:])
                nc.scalar.activation(os1[:], P1[:], AF.Identity, bias=o_b1[:])
                nc.sync.dma_start(out=out_ext[bi, 0:128, cs], in_=os0[:])
                nc.sync.dma_start(out=out_ext[bi, 128:C, cs], in_=os1[:])

        # pipeline: batch-0 q conv on the (idle) PE during prologue; batch-1
        # dw emitted in small chunks between batch-0 attention pairs so
        # pair-critical V/G ops aren't stuck behind long tap chains.
        qcp0 = {"A": emit_dw_q_pe(0, "A"), "B": emit_dw_q_pe(0, "B")}
        emit_dw_kv(0, "k")
        emit_dw_q_cast(0, qcp0)
        emit_dw_kv(0, "v")
        t0 = pw_block(0)
        emit_convert(1)
        e00 = attn_st(0, 0, *t0)
        emit_dw_q_taps(1, "A", 0, 5)
        e01 = attn_st(0, 1, *t0)
        attn_av(0, 0, e00, *t0)
        emit_dw_q_taps(1, "A", 5, 9)
        e02 = attn_st(0, 2, *t0)
        attn_av(0, 1, e01, *t0)
        emit_dw_q_taps(1, "B", 0, 5)
        e03 = attn_st(0, 3, *t0)
        attn_av(0, 2, e02, *t0)
        emit_dw_q_taps(1, "B", 5, 9)
        emit_dw_q_b1_reorder()
        flush_refolds()
        o_proj_half(0, 0)
        emit_dw_kv(1, "k", ("A",))
        attn_av(0, 3, e03, *t0)
        emit_dw_kv(1, "k", ("B",))
        emit_dw_kv(1, "v")
        t1 = pw_block(1)
        flush_refolds()
        o_proj_half(0, 1)
        e10 = attn_st(1, 0, *t1)
        e11 = attn_st(1, 1, *t1)
        attn_av(1, 0, e10, *t1)
        e12 = attn_st(1, 2, *t1)
        attn_av(1, 1, e11, *t1)
        e13 = attn_st(1, 3, *t1)
        attn_av(1, 2, e12, *t1)
        flush_refolds()
        attn_av(1, 3, e13, *t1)
        o_proj_half(1, 0)
        flush_refolds()
        o_proj_half(1, 1, split_tail=True)

    nc.finalize()
    return nc


def _prep_weights(inputs):
    g = lambda k: np.asarray(inputs[k], np.float32)
    w = {}
    for p in ("q", "k", "v"):
        scale = g(p + "_bn_g") / np.sqrt(g(p + "_bn_v") + EPS)
        dww = g(p + "_dw")[:, 0].reshape(C, 9) * scale[:, None]
        biasc = g(p + "_bn_b") - g(p + "_bn_m") * scale
        pwm = g(p + "_pw")[:, :, 0, 0]
        const_row = pwm @ biasc
        w[p + "pwT"] = np.ascontiguousarray(
            np.concatenate([pwm.T, const_row[None, :]], 0)).astype(BF16NP)
        w[p + "dw"] = np.ascontiguousarray(dww)
    dq = w["qdw"]                                     # [C, 9] fused dw weights
    qdA = np.zeros((128, 9, 128), np.float32)
    qdA[np.arange(128), :, np.arange(128)] = dq[0:128]
    qdB = np.zeros((64, 9, 64), np.float32)
    qdB[np.arange(64), :, np.arange(64)] = dq[128:C]
    w["qdiagA"] = np.ascontiguousarray(qdA.reshape(128, 9 * 128)).astype(BF16NP)
    w["qdiagB"] = np.ascontiguousarray(qdB.reshape(64, 9 * 64)).astype(BF16NP)
    owt = g("o_w")[:, :, 0, 0].T                      # [INNER, C]
    w["owt4"] = np.ascontiguousarray(
        owt.reshape(4, 128, C).transpose(1, 0, 2).reshape(128, 4 * C)
    ).astype(BF16NP)
    w["ones128x64"] = np.ones((128, HID), BF16NP)
    w["onesq"] = np.ones((1, BPC, Lq), BF16NP)
    w["oneskv"] = np.ones((1, BPC, Lkv), BF16NP)
    w["ob"] = np.ascontiguousarray(g("o_b")[:, None])
    return w


def kernel(**inputs):
    global _NC, LAST_RESULT
    if _NC is None:
        _NC = _build()
    w = _prep_weights(inputs)
    x = np.ascontiguousarray(
        np.asarray(inputs["x"], np.float32).reshape(B, C, Lq)).astype(BF16NP)
    in_maps = []
    for c in range(NCORES):
        m = {"x": np.ascontiguousarray(x[c * BPC:(c + 1) * BPC])}
        m.update(w)
        in_maps.append(m)
    res = run_bass_kernel_spmd(_NC, in_maps, list(range(NCORES)))
    LAST_RESULT = res
    out = np.concatenate([r["out"] for r in res.results], 0)
    return np.ascontiguousarray(out.reshape(B, C, S, S).astype(np.float32))


# revision 18
# speedup vs baseline: 1.0401x; 1.0401x over previous
"""Trainium2 Bass kernel for nn_Attention_16071767621814.

MobileViT-style attention block: 3x (depthwise3x3 conv + BN + 1x1 pointwise)
for q/k/v, 8-head attention (Lq=1024, Lkv=256, d=64), head-mixing reshape,
1x1 output projection.

Sharding: pure data-parallel over batch (16 batches / 8 cores = 2 per core),
zero collectives.

Per-core design (all layouts [feature-on-partition, token-on-free]):
- x arrives bf16 from host (halves input DMA), staged contiguously then
  pad-copied on Vector (4x copy mode).
- BN folded host-side: scale into dw weights, bias via pw const row.
- batch-0 depthwise q conv as 9 shifted diagonal matmuls on the (otherwise
  idle) PE during the prologue; batch-1 q conv as Vector/GpSimd taps.
- xqb holds the q dw-conv output in NATURAL lq order; the head-mix token
  reorder (i = m*128 + j for lq = j*8 + m) is folded into the q pointwise
  matmul's rhs access pattern (strided rhs stream costs the PE nothing).
- k/v stride-2 convs read the padded tile directly with stride-2 views.
- S^T = k^T q on PE, exp on ScalarE, denominator via ones-lhsT matmuls
  (emitted BEFORE av so the reciprocal overlaps av matmuls).
- normalize fused into the head-mix refold: 4 strided tensor_tensor
  multiplies per head-pair writing F in place, split Vector/GpSimd.
- o_proj as 16 matmuls of N=512,K=128 per batch from F.
- PE warm-up dummy matmuls during the DMA prologue (avoids the ~1.2GHz
  cold p-state on the first ~4us of real matmuls).
"""

import numpy as np
import ml_dtypes
BF16NP = ml_dtypes.bfloat16

from concourse import bass, bacc, tile, mybir
from concourse.bass_utils import run_bass_kernel_spmd

F32 = mybir.dt.float32
BF16 = mybir.dt.bfloat16
AF = mybir.ActivationFunctionType
OP = mybir.AluOpType

NCORES = 8
B, C, S = 16, 192, 32
BPC = B // NCORES          # 2 batches per core
Lq = S * S                 # 1024
Sk = S // 2                # 16
Lkv = Sk * Sk              # 256
HEADS, HID, INNER = 8, 64, 512
EPS = 1e-5
PS = S + 2                 # padded spatial 34
NWARM = 18                 # PE warm-up dummies

_NC = None
LAST_RESULT = None


def _build():
    nc = bacc.Bacc("TRN2", target_bir_lowering=False, debug=False,
                   num_devices=NCORES)

    x_ext = nc.declare_dram_parameter("x", [BPC, C, Lq], BF16, isOutput=False)
    pw_ext = {}
    dw_ext = {}
    for p in ("q", "k", "v"):
        pw_ext[p] = nc.declare_dram_parameter(p + "pwT", [C + 1, INNER], BF16,
                                              isOutput=False)
        dw_ext[p] = nc.declare_dram_parameter(p + "dw", [C, 9], F32,
                                              isOutput=False)
    owt4_ext = nc.declare_dram_parameter("owt4", [128, 4 * C], BF16,
                                         isOutput=False)
    qdA_ext = nc.declare_dram_parameter("qdiagA", [128, 9 * 128], BF16,
                                        isOutput=False)
    qdB_ext = nc.declare_dram_parameter("qdiagB", [64, 9 * 64], BF16,
                                        isOutput=False)
    ones_ext = nc.declare_dram_parameter("ones128x64", [128, HID], BF16,
                                         isOutput=False)
    onesq_ext = nc.declare_dram_parameter("onesq", [1, BPC, Lq], BF16,
                                          isOutput=False)
    oneskv_ext = nc.declare_dram_parameter("oneskv", [1, BPC, Lkv], BF16,
                                           isOutput=False)
    ob_ext = nc.declare_dram_parameter("ob", [C, 1], F32, isOutput=False)
    out_ext = nc.declare_dram_parameter("out", [BPC, C, Lq], F32, isOutput=True)

    TAPS = [(dy, dx) for dy in range(3) for dx in range(3)]

    from contextlib import ExitStack
    with tile.TileContext(nc) as tc, ExitStack() as ctx:
        const = ctx.enter_context(tc.tile_pool(name="const", bufs=1))
        xpool = ctx.enter_context(tc.tile_pool(name="xpool", bufs=1))
        wpool = ctx.enter_context(tc.tile_pool(name="wpool", bufs=2))
        psw = ctx.enter_context(tc.tile_pool(name="psw", bufs=4, space="PSUM"))

        # ---- PE warm-up: dummy matmuls on memset tiles, no ext deps ----
        warmw = const.tile([128, 128], BF16, name="warmw")
        warmr = const.tile([128, 512], BF16, name="warmr")
        nc.vector.memset(warmw[:], 0.0)
        nc.vector.memset(warmr[:], 0.0)
        for _ in range(NWARM):
            wp = psw.tile([128, 512], F32, name="work")
            nc.tensor.matmul(wp[:], warmw[:], warmr[:], start=True, stop=True)

        # ---- x staging DMA (contiguous, fast); batch 0 + diag weights
        # first so the prologue diag-conv isn't starved ----
        xsA = xpool.tile([128, BPC, Lq], BF16, name="xsA")
        xsB = xpool.tile([C - 128, BPC, Lq], BF16, name="xsB")
        nc.sync.dma_start(out=xsA[:, 0, :], in_=x_ext[0, 0:128, :])
        nc.sync.dma_start(out=xsB[:, 0, :], in_=x_ext[0, 128:C, :])
        qdA = const.tile([128, 9, 128], BF16, name="qdA")
        nc.sync.dma_start(out=qdA[:],
                          in_=qdA_ext[:].rearrange("p (t c) -> p t c", t=9))
        qdB = const.tile([64, 9, 64], BF16, name="qdB")
        nc.sync.dma_start(out=qdB[:],
                          in_=qdB_ext[:].rearrange("p (t c) -> p t c", t=9))

        # padded bf16 input tiles (zero borders only; interior overwritten)
        xpA = xpool.tile([128, BPC, PS, PS], BF16, name="xpA")
        xpB = xpool.tile([C - 128, BPC, PS, PS], BF16, name="xpB")
        for xp_t in (xpA, xpB):
            nc.vector.memset(xp_t[:, :, 0:1, :], 0.0)
            nc.vector.memset(xp_t[:, :, S + 1:S + 2, :], 0.0)
            nc.vector.memset(xp_t[:, :, :, 0:1], 0.0)
            nc.vector.memset(xp_t[:, :, :, S + 1:S + 2], 0.0)

        def emit_convert(bi):
            # bf16 pad-copy on Vector (4x copy mode: packed bf16, SBUF only)
            nc.vector.tensor_copy(
                xpA[:, bi, 1:S + 1, 1:S + 1],
                xsA[:, bi].rearrange("p (h w) -> p h w", h=S))
            nc.vector.tensor_copy(
                xpB[:, bi, 1:S + 1, 1:S + 1],
                xsB[:, bi].rearrange("p (h w) -> p h w", h=S))

        emit_convert(0)

        # ---- weights to SBUF ----
        pwA, pwB, dwA, dwB = {}, {}, {}, {}
        for p in ("q", "k", "v"):
            pwA[p] = const.tile([128, INNER], BF16, name=f"pwA{p}")
            pwB[p] = const.tile([C + 1 - 128, INNER], BF16, name=f"pwB{p}")
            nc.sync.dma_start(out=pwA[p][:], in_=pw_ext[p][0:128, :])
            nc.sync.dma_start(out=pwB[p][:], in_=pw_ext[p][128:C + 1, :])
            dwA[p] = const.tile([128, 9], F32, name=f"dwA{p}")
            dwB[p] = const.tile([C - 128, 9], F32, name=f"dwB{p}")
            nc.sync.dma_start(out=dwA[p][:], in_=dw_ext[p][0:128, :])
            nc.sync.dma_start(out=dwB[p][:], in_=dw_ext[p][128:C, :])
        owt4 = const.tile([128, 4, C], BF16, name="owt4")
        nc.sync.dma_start(out=owt4[:],
                          in_=owt4_ext[:].rearrange("p (k c) -> p k c", k=4))
        # batch-1 staging after the weight DMAs (not prologue-critical)
        nc.sync.dma_start(out=xsA[:, 1, :], in_=x_ext[1, 0:128, :])
        nc.sync.dma_start(out=xsB[:, 1, :], in_=x_ext[1, 128:C, :])
        o_b0 = const.tile([128, 1], F32, name="ob0")
        o_b1 = const.tile([C - 128, 1], F32, name="ob1")
        nc.sync.dma_start(out=o_b0[:], in_=ob_ext[0:128, :])
        nc.sync.dma_start(out=o_b1[:], in_=ob_ext[128:C, :])
        ones64 = const.tile([128, HID], BF16, name="ones64")
        nc.sync.dma_start(out=ones64[:], in_=ones_ext[:])

        # ---- dw conv output tiles (bf16), ones row at partition 64 of B ----
        xqb = {"A": xpool.tile([128, BPC, Lq], BF16, name="xqbA"),
               "B": xpool.tile([65, BPC, Lq], BF16, name="xqbB")}
        xkb = {"A": xpool.tile([128, BPC, Lkv], BF16, name="xkbA"),
               "B": xpool.tile([65, BPC, Lkv], BF16, name="xkbB")}
        xvb = {"A": xpool.tile([128, BPC, Lkv], BF16, name="xvbA"),
               "B": xpool.tile([65, BPC, Lkv], BF16, name="xvbB")}
        nc.sync.dma_start(out=xqb["B"][64:65, :, :], in_=onesq_ext[:])
        nc.sync.dma_start(out=xkb["B"][64:65, :, :], in_=oneskv_ext[:])
        nc.sync.dma_start(out=xvb["B"][64:65, :, :], in_=oneskv_ext[:])

        # batch-1 q dw-conv accumulators (natural lq order) + tap scratch
        aq = {"A": xpool.tile([128, Lq], BF16, name="aqA"),
              "B": xpool.tile([64, Lq], BF16, name="aqB")}
        tmpq = {"A": xpool.tile([128, Lq], BF16, name="tmpqA"),
                "B": xpool.tile([64, Lq], BF16, name="tmpqB")}

        def emit_dw_q_taps(bi, tile_sel, lo=0, hi=9):
            # taps for one tile half into aq; tensor_scalar (4x mode) +
            # tensor_tensor add (2x mode) beats 1x scalar_tensor_tensor
            srcT, dwt = ((xpA, dwA["q"]) if tile_sel == "A"
                         else (xpB, dwB["q"]))
            np_ = min(srcT.shape[0], 128)
            outap = aq[tile_sel][0:np_].rearrange("p (h w) -> p h w", h=S)
            tmpap = tmpq[tile_sel][0:np_].rearrange("p (h w) -> p h w", h=S)
            for t in range(lo, hi):
                dy, dx = TAPS[t]
                inap = srcT[0:np_, bi, dy:dy + S, dx:dx + S]
                if t == 0:
                    nc.vector.tensor_scalar(outap, inap, dwt[:, 0:1],
                                            None, OP.mult)
                else:
                    nc.vector.tensor_scalar(tmpap, inap, dwt[:, t:t + 1],
                                            None, OP.mult)
                    nc.vector.tensor_tensor(outap, outap, tmpap, OP.add)

        def emit_dw_q_b1_reorder():
            # aq (natural) -> xqb (head-mix order i = m*128 + j), on Scalar
            # to keep the Vector queue free for the attention chains
            for tl in ("A", "B"):
                np_ = 128 if tl == "A" else 64
                src_ap = aq[tl][0:np_].rearrange("p (j m) -> p m j", m=8)
                dst_ap = xqb[tl][0:np_, 1].rearrange("p (m j) -> p m j", m=8)
                nc.scalar.copy(dst_ap, src_ap)

        def emit_dw_q_pe(bi, tile_sel):
            # depthwise q conv as 9 accumulating diagonal matmuls on PE
            qd, xp_t, np_ = ((qdA, xpA, 128) if tile_sel == "A"
                             else (qdB, xpB, 64))
            qcp = psw.tile([np_, Lq], F32, name="work")
            for n in range(2):
                ns = slice(n * 512, (n + 1) * 512)
                for t, (dy, dx) in enumerate(TAPS):
                    rhs = xp_t[0:np_, bi, dy + 16 * n:dy + 16 * n + 16,
                               dx:dx + S]
                    nc.tensor.matmul(qcp[:, ns], qd[:, t, :], rhs,
                                     start=(t == 0), stop=(t == 8))
            return qcp

        def emit_dw_q_cast(bi, src_tiles):
            # psum -> sbuf bf16 cast fused with the head-mix reorder
            # (strided src costs the copy engine nothing extra)
            for tl, srcT in src_tiles.items():
                np_ = 128 if tl == "A" else 64
                src_ap = srcT[0:np_].rearrange("p (j m) -> p m j", m=8)
                dst_ap = xqb[tl][0:np_, bi].rearrange("p (m j) -> p m j", m=8)
                nc.scalar.copy(dst_ap, src_ap)

        def emit_dw_kv(bi, p, tiles=("A", "B"), eng=None):
            # stride-2 taps reading the padded tile directly
            eng = eng or nc.vector
            dst = xkb if p == "k" else xvb
            for ti, (xpT, dwt) in enumerate(((xpA, dwA[p]), (xpB, dwB[p]))):
                if ("A", "B")[ti] not in tiles:
                    continue
                np_ = min(xpT.shape[0], 128)
                d = dst["A"] if ti == 0 else dst["B"]
                outap = d[0:np_, bi].rearrange("p (i j) -> p i j", i=Sk)
                for t, (dy, dx) in enumerate(TAPS):
                    inap = xpT[0:np_, bi, dy:dy + 2 * Sk:2, dx:dx + 2 * Sk:2]
                    if t == 0:
                        eng.tensor_scalar(outap, inap, dwt[:, 0:1],
                                          None, OP.mult)
                    else:
                        eng.scalar_tensor_tensor(
                            outap, inap, dwt[:, t:t + 1], outap,
                            OP.mult, OP.add)

        # ---- per-batch compute; q mms first so PE isn't gated on k/v taps --
        def pw_block(bi):
            q_sb, k_sb, vT_sb = [], [], []
            for hp in range(4):
                hs = slice(hp * 128, (hp + 1) * 128)
                qp = psw.tile([128, Lq], F32, name="work")
                for n in range(2):
                    ns = slice(n * 512, (n + 1) * 512)
                    nc.tensor.matmul(qp[:, ns], pwA["q"][:, hs],
                                     xqb["A"][:, bi, ns],
                                     start=True, stop=False)
                    nc.tensor.matmul(qp[:, ns], pwB["q"][:, hs],
                                     xqb["B"][:, bi, ns],
                                     start=False, stop=True)
                qs = wpool.tile([128, Lq], BF16, name=f"qsb{hp}", bufs=2)
                nc.scalar.copy(qs[:], qp[:])
                q_sb.append(qs)
            for hp in range(4):
                hs = slice(hp * 128, (hp + 1) * 128)
                kp = psw.tile([128, Lkv], F32, name="work")
                nc.tensor.matmul(kp[:], pwA["k"][:, hs],
                                 xkb["A"][:, bi, :],
                                 start=True, stop=False)
                nc.tensor.matmul(kp[:], pwB["k"][:, hs],
                                 xkb["B"][:, bi, :],
                                 start=False, stop=True)
                ks = wpool.tile([128, Lkv], BF16, name=f"ksb{hp}", bufs=2)
                nc.scalar.copy(ks[:], kp[:])
                k_sb.append(ks)
            for kc in range(2):
                vp = psw.tile([128, INNER], F32, name="work")
                kvs = slice(kc * 128, (kc + 1) * 128)
                nc.tensor.matmul(vp[:], xvb["A"][:, bi, kvs],
                                 pwA["v"][:], start=True, stop=False)
                nc.tensor.matmul(vp[:], xvb["B"][:, bi, kvs],
                                 pwB["v"][:], start=False, stop=True)
                vs = wpool.tile([128, INNER], BF16, name=f"vtsb{kc}", bufs=2)
                nc.scalar.copy(vs[:], vp[:])
                vT_sb.append(vs)
            return q_sb, k_sb, vT_sb

        Fs = {}

        def attn_st(bi, hp, q_sb, k_sb, vT_sb):
            if hp == 0:
                Fs[bi] = wpool.tile([128, 4, Lq], BF16, name="Ftile", bufs=2)
            h0, h1 = 2 * hp, 2 * hp + 1
            qs_t, ks_t = q_sb[hp], k_sb[hp]
            expAs = {}
            for idx, h in enumerate((h0, h1)):
                R = slice(idx * HID, (idx + 1) * HID)
                for kc in range(2):
                    st = psw.tile([128, Lq], F32, name="work")
                    kvs = slice(kc * 128, (kc + 1) * 128)
                    for n in range(2):
                        ns = slice(n * 512, (n + 1) * 512)
                        nc.tensor.matmul(st[:, ns],
                                         ks_t[R, kvs],
                                         qs_t[R, ns],
                                         start=True, stop=True)
                    ex = wpool.tile([128, Lq], BF16, name="expA", bufs=8)
                    nc.scalar.activation(ex[:], st[:], AF.Exp,
                                         scale=1.0 / (HID ** 0.5))
                    expAs[(h, kc)] = ex
            return expAs

        def attn_av(bi, hp, expAs, q_sb, k_sb, vT_sb):
            F = Fs[bi]
            h0, h1 = 2 * hp, 2 * hp + 1
            # dnp allocated first so pw/o_proj psum slots recycle onto the
            # recip-freed dnp rather than the late-freed avp
            dnp = psw.tile([128, Lq], F32, name="work")
            avp = psw.tile([128, Lq], F32, name="work")
            # denominator matmuls first: the reciprocal overlaps av matmuls
            for idx, h in enumerate((h0, h1)):
                tp = (0, 0) if idx == 0 else (0, 64)
                rows = slice(idx * HID, (idx + 1) * HID)
                for kc in range(2):
                    for n in range(2):
                        ns = slice(n * 512, (n + 1) * 512)
                        nc.tensor.matmul(dnp[rows, ns], ones64[:, :],
                                         expAs[(h, kc)][:, ns],
                                         start=(kc == 0), stop=(kc == 1),
                                         tile_position=tp)
            rc = wpool.tile([128, Lq], F32, name="recip", bufs=2)
            nc.vector.reciprocal_approx_fast(rc[:], dnp[:])
            for idx, h in enumerate((h0, h1)):
                tp = (0, 0) if idx == 0 else (0, 64)
                rows = slice(idx * HID, (idx + 1) * HID)
                hs = slice(h * HID, (h + 1) * HID)
                for kc in range(2):
                    for n in range(2):
                        ns = slice(n * 512, (n + 1) * 512)
                        nc.tensor.matmul(avp[rows, ns],
                                         vT_sb[kc][:, hs],
                                         expAs[(h, kc)][:, ns],
                                         start=(kc == 0), stop=(kc == 1),
                                         tile_position=tp)
            # normalize (V) then head-mix refold split Vector/Scalar
            avbf = wpool.tile([128, Lq], BF16, name="avbf", bufs=2)
            nc.vector.tensor_tensor(avbf[:], avp[:], rc[:], OP.mult)
            for idx, h in enumerate((h0, h1)):
                rows = slice(idx * HID, (idx + 1) * HID)
                src4 = avbf[rows, :].rearrange("p (a b j) -> p a b j",
                                               a=4, b=2)
                for par in range(2):
                    dst = F[par * HID:(par + 1) * HID, :,
                            h * 128:(h + 1) * 128]
                    if par == 0:
                        nc.vector.tensor_copy(dst, src4[:, :, par, :])
                    else:
                        nc.scalar.copy(dst, src4[:, :, par, :])

        def o_proj_half(bi, half, split_tail=False):
            cs = slice(half * 512, (half + 1) * 512)
            P0 = psw.tile([128, 512], F32, name="work")
            P1 = psw.tile([C - 128, 512], F32, name="work")
            for kk in range(4):
                nc.tensor.matmul(P0[:], owt4[:, kk, 0:128],
                                 Fs[bi][:, kk, cs],
                                 start=(kk == 0), stop=(kk == 3))
                nc.tensor.matmul(P1[:], owt4[:, kk, 128:C],
                                 Fs[bi][:, kk, cs],
                                 start=(kk == 0), stop=(kk == 3))
            os0 = wpool.tile([128, 512], F32, name="os0", bufs=2)
            os1 = wpool.tile([C - 128, 512], F32, name="os1", bufs=2)
            if split_tail:
                for c0 in range(0, 512, 256):
                    cc = slice(c0, c0 + 256)
                    nc.scalar.activation(os0[:, cc], P0[:, cc], AF.Identity,
                                         bias=o_b0[:])
                    nc.vector.tensor_scalar(os1[:, cc], P1[:, cc], o_b1[:],
                                            None, OP.add)
                    nc.sync.dma_start(out=out_ext[bi, 0:128, half * 512 + c0:
                                                  half * 512 + c0 + 256],
                                      in_=os0[:, cc])
                    nc.sync.dma_start(out=out_ext[bi, 128:C, half * 512 + c0:
                                                  half * 512 + c0 + 256],
                                      in_=os1[:, cc])
            else:
                nc.scalar.activation(os0[:], P0[:], AF.Identity, bias=o_b0[:])
                nc.scalar.activation(os1[:], P1[:], AF.Identity, bias=o_b1[:])
                nc.sync.dma_start(out=out_ext[bi, 0:128, cs], in_=os0[:])
                nc.sync.dma_start(out=out_ext[bi, 128:C, cs], in_=os1[:])

        # pipeline: batch-0 q conv on the (idle) PE during prologue; batch-1
        # dw emitted in small chunks between batch-0 attention pairs so
        # pair-critical V/G ops aren't stuck behind long tap chains.
        qcp0 = {"A": emit_dw_q_pe(0, "A"), "B": emit_dw_q_pe(0, "B")}
        emit_dw_kv(0, "k")
        emit_dw_q_cast(0, qcp0)
        emit_dw_kv(0, "v")
        t0 = pw_block(0)
        emit_convert(1)
        e00 = attn_st(0, 0, *t0)
        emit_dw_q_taps(1, "A", 0, 5)
        e01 = attn_st(0, 1, *t0)
        attn_av(0, 0, e00, *t0)
        emit_dw_q_taps(1, "A", 5, 9)
        e02 = attn_st(0, 2, *t0)
        attn_av(0, 1, e01, *t0)
        emit_dw_q_taps(1, "B", 0, 5)
        e03 = attn_st(0, 3, *t0)
        attn_av(0, 2, e02, *t0)
        emit_dw_q_taps(1, "B", 5, 9)
        emit_dw_q_b1_reorder()
        o_proj_half(0, 0)
        emit_dw_kv(1, "k", ("A",))
        attn_av(0, 3, e03, *t0)
        emit_dw_kv(1, "k", ("B",))
        emit_dw_kv(1, "v")
        t1 = pw_block(1)
        o_proj_half(0, 1)
        e10 = attn_st(1, 0, *t1)
        e11 = attn_st(1, 1, *t1)
        attn_av(1, 0, e10, *t1)
        e12 = attn_st(1, 2, *t1)
        attn_av(1, 1, e11, *t1)
        e13 = attn_st(1, 3, *t1)
        attn_av(1, 2, e12, *t1)
        attn_av(1, 3, e13, *t1)
        o_proj_half(1, 0)
        o_proj_half(1, 1, split_tail=True)

    nc.finalize()
    return nc


def _prep_weights(inputs):
    g = lambda k: np.asarray(inputs[k], np.float32)
    w = {}
    for p in ("q", "k", "v"):
        scale = g(p + "_bn_g") / np.sqrt(g(p + "_bn_v") + EPS)
        dww = g(p + "_dw")[:, 0].reshape(C, 9) * scale[:, None]
        biasc = g(p + "_bn_b") - g(p + "_bn_m") * scale
        pwm = g(p + "_pw")[:, :, 0, 0]
        const_row = pwm @ biasc
        w[p + "pwT"] = np.ascontiguousarray(
            np.concatenate([pwm.T, const_row[None, :]], 0)).astype(BF16NP)
        w[p + "dw"] = np.ascontiguousarray(dww)
    dq = w["qdw"]                                     # [C, 9] fused dw weights
    qdA = np.zeros((128, 9, 128), np.float32)
    qdA[np.arange(128), :, np.arange(128)] = dq[0:128]
    qdB = np.zeros((64, 9, 64), np.float32)
    qdB[np.arange(64), :, np.arange(64)] = dq[128:C]
    w["qdiagA"] = np.ascontiguousarray(qdA.reshape(128, 9 * 128)).astype(BF16NP)
    w["qdiagB"] = np.ascontiguousarray(qdB.reshape(64, 9 * 64)).astype(BF16NP)
    owt = g("o_w")[:, :, 0, 0].T                      # [INNER, C]
    w["owt4"] = np.ascontiguousarray(
        owt.reshape(4, 128, C).transpose(1, 0, 2).reshape(128, 4 * C)
    ).astype(BF16NP)
    w["ones128x64"] = np.ones((128, HID), BF16NP)
    w["onesq"] = np.ones((1, BPC, Lq), BF16NP)
    w["oneskv"] = np.ones((1, BPC, Lkv), BF16NP)
    w["ob"] = np.ascontiguousarray(g("o_b")[:, None])
    return w


def kernel(**inputs):
    global _NC, LAST_RESULT
    if _NC is None:
        _NC = _build()
    w = _prep_weights(inputs)
    x = np.ascontiguousarray(
        np.asarray(inputs["x"], np.float32).reshape(B, C, Lq)).astype(BF16NP)
    in_maps = []
    for c in range(NCORES):
        m = {"x": np.ascontiguousarray(x[c * BPC:(c + 1) * BPC])}
        m.update(w)
        in_maps.append(m)
    res = run_bass_kernel_spmd(_NC, in_maps, list(range(NCORES)))
    LAST_RESULT = res
    out = np.concatenate([r["out"] for r in res.results], 0)
    return np.ascontiguousarray(out.reshape(B, C, S, S).astype(np.float32))


# revision 19
# speedup vs baseline: 1.0451x; 1.0048x over previous
"""Trainium2 Bass kernel for nn_Attention_16071767621814.

MobileViT-style attention block: 3x (depthwise3x3 conv + BN + 1x1 pointwise)
for q/k/v, 8-head attention (Lq=1024, Lkv=256, d=64), head-mixing reshape,
1x1 output projection.

Sharding: pure data-parallel over batch (16 batches / 8 cores = 2 per core),
zero collectives.

Per-core design (all layouts [feature-on-partition, token-on-free]):
- x arrives bf16 from host (halves input DMA), staged contiguously then
  pad-copied on Vector (4x copy mode).
- BN folded host-side: scale into dw weights, bias via pw const row.
- batch-0 depthwise q conv as 9 shifted diagonal matmuls on the (otherwise
  idle) PE during the prologue; batch-1 q conv as Vector/GpSimd taps.
- xqb holds the q dw-conv output in NATURAL lq order; the head-mix token
  reorder (i = m*128 + j for lq = j*8 + m) is folded into the q pointwise
  matmul's rhs access pattern (strided rhs stream costs the PE nothing).
- k/v stride-2 convs read the padded tile directly with stride-2 views.
- S^T = k^T q on PE, exp on ScalarE, denominator via ones-lhsT matmuls
  (emitted BEFORE av so the reciprocal overlaps av matmuls).
- normalize fused into the head-mix refold: 4 strided tensor_tensor
  multiplies per head-pair writing F in place, split Vector/GpSimd.
- o_proj as 16 matmuls of N=512,K=128 per batch from F.
- PE warm-up dummy matmuls during the DMA prologue (avoids the ~1.2GHz
  cold p-state on the first ~4us of real matmuls).
"""

import numpy as np
import ml_dtypes
BF16NP = ml_dtypes.bfloat16

from concourse import bass, bacc, tile, mybir
from concourse.bass_utils import run_bass_kernel_spmd

F32 = mybir.dt.float32
BF16 = mybir.dt.bfloat16
AF = mybir.ActivationFunctionType
OP = mybir.AluOpType

NCORES = 8
B, C, S = 16, 192, 32
BPC = B // NCORES          # 2 batches per core
Lq = S * S                 # 1024
Sk = S // 2                # 16
Lkv = Sk * Sk              # 256
HEADS, HID, INNER = 8, 64, 512
EPS = 1e-5
PS = S + 2                 # padded spatial 34
NWARM = 9                  # PE warm-up dummies

_NC = None
LAST_RESULT = None


def _build():
    nc = bacc.Bacc("TRN2", target_bir_lowering=False, debug=False,
                   num_devices=NCORES)

    x_ext = nc.declare_dram_parameter("x", [BPC, C, Lq], BF16, isOutput=False)
    pw_ext = {}
    dw_ext = {}
    for p in ("q", "k", "v"):
        pw_ext[p] = nc.declare_dram_parameter(p + "pwT", [C + 1, INNER], BF16,
                                              isOutput=False)
        dw_ext[p] = nc.declare_dram_parameter(p + "dw", [C, 9], F32,
                                              isOutput=False)
    owt4_ext = nc.declare_dram_parameter("owt4", [128, 4 * C], BF16,
                                         isOutput=False)
    qdA_ext = nc.declare_dram_parameter("qdiagA", [128, 9 * 128], BF16,
                                        isOutput=False)
    qdB_ext = nc.declare_dram_parameter("qdiagB", [64, 9 * 64], BF16,
                                        isOutput=False)
    ones_ext = nc.declare_dram_parameter("ones128x64", [128, HID], BF16,
                                         isOutput=False)
    onesq_ext = nc.declare_dram_parameter("onesq", [1, BPC, Lq], BF16,
                                          isOutput=False)
    oneskv_ext = nc.declare_dram_parameter("oneskv", [1, BPC, Lkv], BF16,
                                           isOutput=False)
    ob_ext = nc.declare_dram_parameter("ob", [C, 1], F32, isOutput=False)
    out_ext = nc.declare_dram_parameter("out", [BPC, C, Lq], F32, isOutput=True)

    TAPS = [(dy, dx) for dy in range(3) for dx in range(3)]

    from contextlib import ExitStack
    with tile.TileContext(nc) as tc, ExitStack() as ctx:
        const = ctx.enter_context(tc.tile_pool(name="const", bufs=1))
        xpool = ctx.enter_context(tc.tile_pool(name="xpool", bufs=1))
        wpool = ctx.enter_context(tc.tile_pool(name="wpool", bufs=2))
        psw = ctx.enter_context(tc.tile_pool(name="psw", bufs=4, space="PSUM"))

        # ---- PE warm-up: dummy K=1 matmuls on tiny memset tiles (the
        # stream length, not K, ramps the p-state) ----
        warmw = const.tile([1, 128], BF16, name="warmw")
        warmr = const.tile([1, 512], BF16, name="warmr")
        nc.vector.memset(warmw[:], 0.0)
        nc.vector.memset(warmr[:], 0.0)
        for _ in range(NWARM):
            wp = psw.tile([128, 512], F32, name="work")
            nc.tensor.matmul(wp[:], warmw[:], warmr[:], start=True, stop=True)

        # ---- x staging DMA (contiguous, fast); batch 0 + diag weights
        # first so the prologue diag-conv isn't starved ----
        xsA = xpool.tile([128, BPC, Lq], BF16, name="xsA")
        xsB = xpool.tile([C - 128, BPC, Lq], BF16, name="xsB")
        nc.sync.dma_start(out=xsA[:, 0, :], in_=x_ext[0, 0:128, :])
        nc.sync.dma_start(out=xsB[:, 0, :], in_=x_ext[0, 128:C, :])
        qdA = const.tile([128, 9, 128], BF16, name="qdA")
        nc.sync.dma_start(out=qdA[:],
                          in_=qdA_ext[:].rearrange("p (t c) -> p t c", t=9))
        qdB = const.tile([64, 9, 64], BF16, name="qdB")
        nc.sync.dma_start(out=qdB[:],
                          in_=qdB_ext[:].rearrange("p (t c) -> p t c", t=9))

        # padded bf16 input tiles (zero borders only; interior overwritten)
        xpA = xpool.tile([128, BPC, PS, PS], BF16, name="xpA")
        xpB = xpool.tile([C - 128, BPC, PS, PS], BF16, name="xpB")
        for xp_t in (xpA, xpB):
            nc.vector.memset(xp_t[:, :, 0:1, :], 0.0)
            nc.vector.memset(xp_t[:, :, S + 1:S + 2, :], 0.0)
            nc.vector.memset(xp_t[:, :, :, 0:1], 0.0)
            nc.vector.memset(xp_t[:, :, :, S + 1:S + 2], 0.0)

        def emit_convert(bi):
            # bf16 pad-copy on Vector (4x copy mode: packed bf16, SBUF only)
            nc.vector.tensor_copy(
                xpA[:, bi, 1:S + 1, 1:S + 1],
                xsA[:, bi].rearrange("p (h w) -> p h w", h=S))
            nc.vector.tensor_copy(
                xpB[:, bi, 1:S + 1, 1:S + 1],
                xsB[:, bi].rearrange("p (h w) -> p h w", h=S))

        emit_convert(0)

        # ---- weights to SBUF ----
        pwA, pwB, dwA, dwB = {}, {}, {}, {}
        for p in ("q", "k", "v"):
            pwA[p] = const.tile([128, INNER], BF16, name=f"pwA{p}")
            pwB[p] = const.tile([C + 1 - 128, INNER], BF16, name=f"pwB{p}")
            nc.sync.dma_start(out=pwA[p][:], in_=pw_ext[p][0:128, :])
            nc.sync.dma_start(out=pwB[p][:], in_=pw_ext[p][128:C + 1, :])
            dwA[p] = const.tile([128, 9], F32, name=f"dwA{p}")
            dwB[p] = const.tile([C - 128, 9], F32, name=f"dwB{p}")
            nc.sync.dma_start(out=dwA[p][:], in_=dw_ext[p][0:128, :])
            nc.sync.dma_start(out=dwB[p][:], in_=dw_ext[p][128:C, :])
        owt4 = const.tile([128, 4, C], BF16, name="owt4")
        nc.sync.dma_start(out=owt4[:],
                          in_=owt4_ext[:].rearrange("p (k c) -> p k c", k=4))
        # batch-1 staging after the weight DMAs (not prologue-critical)
        nc.sync.dma_start(out=xsA[:, 1, :], in_=x_ext[1, 0:128, :])
        nc.sync.dma_start(out=xsB[:, 1, :], in_=x_ext[1, 128:C, :])
        o_b0 = const.tile([128, 1], F32, name="ob0")
        o_b1 = const.tile([C - 128, 1], F32, name="ob1")
        nc.sync.dma_start(out=o_b0[:], in_=ob_ext[0:128, :])
        nc.sync.dma_start(out=o_b1[:], in_=ob_ext[128:C, :])
        ones64 = const.tile([128, HID], BF16, name="ones64")
        nc.sync.dma_start(out=ones64[:], in_=ones_ext[:])

        # ---- dw conv output tiles (bf16), ones row at partition 64 of B ----
        xqb = {"A": xpool.tile([128, BPC, Lq], BF16, name="xqbA"),
               "B": xpool.tile([65, BPC, Lq], BF16, name="xqbB")}
        xkb = {"A": xpool.tile([128, BPC, Lkv], BF16, name="xkbA"),
               "B": xpool.tile([65, BPC, Lkv], BF16, name="xkbB")}
        xvb = {"A": xpool.tile([128, BPC, Lkv], BF16, name="xvbA"),
               "B": xpool.tile([65, BPC, Lkv], BF16, name="xvbB")}
        nc.sync.dma_start(out=xqb["B"][64:65, :, :], in_=onesq_ext[:])
        nc.sync.dma_start(out=xkb["B"][64:65, :, :], in_=oneskv_ext[:])
        nc.sync.dma_start(out=xvb["B"][64:65, :, :], in_=oneskv_ext[:])

        # batch-1 q dw-conv accumulators (natural lq order) + tap scratch
        aq = {"A": xpool.tile([128, Lq], BF16, name="aqA"),
              "B": xpool.tile([64, Lq], BF16, name="aqB")}
        tmpq = {"A": xpool.tile([128, Lq], BF16, name="tmpqA"),
                "B": xpool.tile([64, Lq], BF16, name="tmpqB")}

        def emit_dw_q_taps(bi, tile_sel, lo=0, hi=9):
            # taps for one tile half into aq; tensor_scalar (4x mode) +
            # tensor_tensor add (2x mode) beats 1x scalar_tensor_tensor
            srcT, dwt = ((xpA, dwA["q"]) if tile_sel == "A"
                         else (xpB, dwB["q"]))
            np_ = min(srcT.shape[0], 128)
            outap = aq[tile_sel][0:np_].rearrange("p (h w) -> p h w", h=S)
            tmpap = tmpq[tile_sel][0:np_].rearrange("p (h w) -> p h w", h=S)
            for t in range(lo, hi):
                dy, dx = TAPS[t]
                inap = srcT[0:np_, bi, dy:dy + S, dx:dx + S]
                if t == 0:
                    nc.vector.tensor_scalar(outap, inap, dwt[:, 0:1],
                                            None, OP.mult)
                else:
                    nc.vector.tensor_scalar(tmpap, inap, dwt[:, t:t + 1],
                                            None, OP.mult)
                    nc.vector.tensor_tensor(outap, outap, tmpap, OP.add)

        def emit_dw_q_b1_reorder():
            # aq (natural) -> xqb (head-mix order i = m*128 + j), on Scalar
            # to keep the Vector queue free for the attention chains
            for tl in ("A", "B"):
                np_ = 128 if tl == "A" else 64
                src_ap = aq[tl][0:np_].rearrange("p (j m) -> p m j", m=8)
                dst_ap = xqb[tl][0:np_, 1].rearrange("p (m j) -> p m j", m=8)
                nc.scalar.copy(dst_ap, src_ap)

        def emit_dw_q_pe(bi, tile_sel):
            # depthwise q conv as 9 accumulating diagonal matmuls on PE
            qd, xp_t, np_ = ((qdA, xpA, 128) if tile_sel == "A"
                             else (qdB, xpB, 64))
            qcp = psw.tile([np_, Lq], F32, name="work")
            for n in range(2):
                ns = slice(n * 512, (n + 1) * 512)
                for t, (dy, dx) in enumerate(TAPS):
                    rhs = xp_t[0:np_, bi, dy + 16 * n:dy + 16 * n + 16,
                               dx:dx + S]
                    nc.tensor.matmul(qcp[:, ns], qd[:, t, :], rhs,
                                     start=(t == 0), stop=(t == 8))
            return qcp

        def emit_dw_q_cast(bi, src_tiles):
            # psum -> sbuf bf16 cast fused with the head-mix reorder
            # (strided src costs the copy engine nothing extra)
            for tl, srcT in src_tiles.items():
                np_ = 128 if tl == "A" else 64
                src_ap = srcT[0:np_].rearrange("p (j m) -> p m j", m=8)
                dst_ap = xqb[tl][0:np_, bi].rearrange("p (m j) -> p m j", m=8)
                nc.scalar.copy(dst_ap, src_ap)

        def emit_dw_kv(bi, p, tiles=("A", "B"), eng=None):
            # stride-2 taps reading the padded tile directly
            eng = eng or nc.vector
            dst = xkb if p == "k" else xvb
            for ti, (xpT, dwt) in enumerate(((xpA, dwA[p]), (xpB, dwB[p]))):
                if ("A", "B")[ti] not in tiles:
                    continue
                np_ = min(xpT.shape[0], 128)
                d = dst["A"] if ti == 0 else dst["B"]
                outap = d[0:np_, bi].rearrange("p (i j) -> p i j", i=Sk)
                for t, (dy, dx) in enumerate(TAPS):
                    inap = xpT[0:np_, bi, dy:dy + 2 * Sk:2, dx:dx + 2 * Sk:2]
                    if t == 0:
                        eng.tensor_scalar(outap, inap, dwt[:, 0:1],
                                          None, OP.mult)
                    else:
                        eng.scalar_tensor_tensor(
                            outap, inap, dwt[:, t:t + 1], outap,
                            OP.mult, OP.add)

        # ---- per-batch compute; q mms first so PE isn't gated on k/v taps --
        def pw_block(bi):
            q_sb, k_sb, vT_sb = [], [], []
            for hp in range(4):
                hs = slice(hp * 128, (hp + 1) * 128)
                qp = psw.tile([128, Lq], F32, name="work")
                for n in range(2):
                    ns = slice(n * 512, (n + 1) * 512)
                    nc.tensor.matmul(qp[:, ns], pwA["q"][:, hs],
                                     xqb["A"][:, bi, ns],
                                     start=True, stop=False)
                    nc.tensor.matmul(qp[:, ns], pwB["q"][:, hs],
                                     xqb["B"][:, bi, ns],
                                     start=False, stop=True)
                qs = wpool.tile([128, Lq], BF16, name=f"qsb{hp}", bufs=2)
                nc.scalar.copy(qs[:], qp[:])
                q_sb.append(qs)
            for hp in range(4):
                hs = slice(hp * 128, (hp + 1) * 128)
                kp = psw.tile([128, Lkv], F32, name="work")
                nc.tensor.matmul(kp[:], pwA["k"][:, hs],
                                 xkb["A"][:, bi, :],
                                 start=True, stop=False)
                nc.tensor.matmul(kp[:], pwB["k"][:, hs],
                                 xkb["B"][:, bi, :],
                                 start=False, stop=True)
                ks = wpool.tile([128, Lkv], BF16, name=f"ksb{hp}", bufs=2)
                nc.scalar.copy(ks[:], kp[:])
                k_sb.append(ks)
            for kc in range(2):
                vp = psw.tile([128, INNER], F32, name="work")
                kvs = slice(kc * 128, (kc + 1) * 128)
                nc.tensor.matmul(vp[:], xvb["A"][:, bi, kvs],
                                 pwA["v"][:], start=True, stop=False)
                nc.tensor.matmul(vp[:], xvb["B"][:, bi, kvs],
                                 pwB["v"][:], start=False, stop=True)
                vs = wpool.tile([128, INNER], BF16, name=f"vtsb{kc}", bufs=2)
                nc.scalar.copy(vs[:], vp[:])
                vT_sb.append(vs)
            return q_sb, k_sb, vT_sb

        Fs = {}

        def attn_st(bi, hp, q_sb, k_sb, vT_sb):
            if hp == 0:
                Fs[bi] = wpool.tile([128, 4, Lq], BF16, name="Ftile", bufs=2)
            h0, h1 = 2 * hp, 2 * hp + 1
            qs_t, ks_t = q_sb[hp], k_sb[hp]
            expAs = {}
            for idx, h in enumerate((h0, h1)):
                R = slice(idx * HID, (idx + 1) * HID)
                for kc in range(2):
                    st = psw.tile([128, Lq], F32, name="work")
                    kvs = slice(kc * 128, (kc + 1) * 128)
                    for n in range(2):
                        ns = slice(n * 512, (n + 1) * 512)
                        nc.tensor.matmul(st[:, ns],
                                         ks_t[R, kvs],
                                         qs_t[R, ns],
                                         start=True, stop=True)
                    ex = wpool.tile([128, Lq], BF16, name="expA", bufs=8)
                    nc.scalar.activation(ex[:], st[:], AF.Exp,
                                         scale=1.0 / (HID ** 0.5))
                    expAs[(h, kc)] = ex
            return expAs

        def attn_av(bi, hp, expAs, q_sb, k_sb, vT_sb):
            F = Fs[bi]
            h0, h1 = 2 * hp, 2 * hp + 1
            # dnp allocated first so pw/o_proj psum slots recycle onto the
            # recip-freed dnp rather than the late-freed avp
            dnp = psw.tile([128, Lq], F32, name="work")
            avp = psw.tile([128, Lq], F32, name="work")
            # denominator matmuls first: the reciprocal overlaps av matmuls
            for idx, h in enumerate((h0, h1)):
                tp = (0, 0) if idx == 0 else (0, 64)
                rows = slice(idx * HID, (idx + 1) * HID)
                for kc in range(2):
                    for n in range(2):
                        ns = slice(n * 512, (n + 1) * 512)
                        nc.tensor.matmul(dnp[rows, ns], ones64[:, :],
                                         expAs[(h, kc)][:, ns],
                                         start=(kc == 0), stop=(kc == 1),
                                         tile_position=tp)
            rc = wpool.tile([128, Lq], F32, name="recip", bufs=2)
            nc.vector.reciprocal_approx_fast(rc[:], dnp[:])
            for idx, h in enumerate((h0, h1)):
                tp = (0, 0) if idx == 0 else (0, 64)
                rows = slice(idx * HID, (idx + 1) * HID)
                hs = slice(h * HID, (h + 1) * HID)
                for kc in range(2):
                    for n in range(2):
                        ns = slice(n * 512, (n + 1) * 512)
                        nc.tensor.matmul(avp[rows, ns],
                                         vT_sb[kc][:, hs],
                                         expAs[(h, kc)][:, ns],
                                         start=(kc == 0), stop=(kc == 1),
                                         tile_position=tp)
            # normalize (V) then head-mix refold split Vector/Scalar
            avbf = wpool.tile([128, Lq], BF16, name="avbf", bufs=2)
            nc.vector.tensor_tensor(avbf[:], avp[:], rc[:], OP.mult)
            for idx, h in enumerate((h0, h1)):
                rows = slice(idx * HID, (idx + 1) * HID)
                src4 = avbf[rows, :].rearrange("p (a b j) -> p a b j",
                                               a=4, b=2)
                for par in range(2):
                    dst = F[par * HID:(par + 1) * HID, :,
                            h * 128:(h + 1) * 128]
                    if par == 0:
                        nc.vector.tensor_copy(dst, src4[:, :, par, :])
                    else:
                        nc.scalar.copy(dst, src4[:, :, par, :])

        def o_proj_half(bi, half, split_tail=False):
            cs = slice(half * 512, (half + 1) * 512)
            P0 = psw.tile([128, 512], F32, name="work")
            P1 = psw.tile([C - 128, 512], F32, name="work")
            for kk in range(4):
                nc.tensor.matmul(P0[:], owt4[:, kk, 0:128],
                                 Fs[bi][:, kk, cs],
                                 start=(kk == 0), stop=(kk == 3))
                nc.tensor.matmul(P1[:], owt4[:, kk, 128:C],
                                 Fs[bi][:, kk, cs],
                                 start=(kk == 0), stop=(kk == 3))
            os0 = wpool.tile([128, 512], F32, name="os0", bufs=2)
            os1 = wpool.tile([C - 128, 512], F32, name="os1", bufs=2)
            if split_tail:
                for c0 in range(0, 512, 256):
                    cc = slice(c0, c0 + 256)
                    nc.scalar.activation(os0[:, cc], P0[:, cc], AF.Identity,
                                         bias=o_b0[:])
                    nc.vector.tensor_scalar(os1[:, cc], P1[:, cc], o_b1[:],
                                            None, OP.add)
                    nc.sync.dma_start(out=out_ext[bi, 0:128, half * 512 + c0:
                                                  half * 512 + c0 + 256],
                                      in_=os0[:, cc])
                    nc.sync.dma_start(out=out_ext[bi, 128:C, half * 512 + c0:
                                                  half * 512 + c0 + 256],
                                      in_=os1[:, cc])
            else:
                nc.scalar.activation(os0[:], P0[:], AF.Identity, bias=o_b0[:])
                nc.scalar.activation(os1[:], P1[:], AF.Identity, bias=o_b1[:])
                nc.sync.dma_start(out=out_ext[bi, 0:128, cs], in_=os0[:])
                nc.sync.dma_start(out=out_ext[bi, 128:C, cs], in_=os1[:])

        # pipeline: batch-0 q conv on the (idle) PE during prologue; batch-1
        # dw emitted in small chunks between batch-0 attention pairs so
        # pair-critical V/G ops aren't stuck behind long tap chains.
        qcp0 = {"A": emit_dw_q_pe(0, "A"), "B": emit_dw_q_pe(0, "B")}
        emit_dw_kv(0, "k")
        emit_dw_q_cast(0, qcp0)
        emit_dw_kv(0, "v")
        t0 = pw_block(0)
        emit_convert(1)
        e00 = attn_st(0, 0, *t0)
        emit_dw_q_taps(1, "A", 0, 5)
        e01 = attn_st(0, 1, *t0)
        attn_av(0, 0, e00, *t0)
        emit_dw_q_taps(1, "A", 5, 9)
        e02 = attn_st(0, 2, *t0)
        attn_av(0, 1, e01, *t0)
        emit_dw_q_taps(1, "B", 0, 5)
        e03 = attn_st(0, 3, *t0)
        attn_av(0, 2, e02, *t0)
        emit_dw_q_taps(1, "B", 5, 9)
        emit_dw_q_b1_reorder()
        o_proj_half(0, 0)
        emit_dw_kv(1, "k", ("A",))
        attn_av(0, 3, e03, *t0)
        emit_dw_kv(1, "k", ("B",))
        emit_dw_kv(1, "v")
        t1 = pw_block(1)
        o_proj_half(0, 1)
        e10 = attn_st(1, 0, *t1)
        e11 = attn_st(1, 1, *t1)
        attn_av(1, 0, e10, *t1)
        e12 = attn_st(1, 2, *t1)
        attn_av(1, 1, e11, *t1)
        e13 = attn_st(1, 3, *t1)
        attn_av(1, 2, e12, *t1)
        attn_av(1, 3, e13, *t1)
        o_proj_half(1, 0)
        o_proj_half(1, 1, split_tail=True)

    nc.finalize()
    return nc


def _prep_weights(inputs):
    g = lambda k: np.asarray(inputs[k], np.float32)
    w = {}
    for p in ("q", "k", "v"):
        scale = g(p + "_bn_g") / np.sqrt(g(p + "_bn_v") + EPS)
        dww = g(p + "_dw")[:, 0].reshape(C, 9) * scale[:, None]
        biasc = g(p + "_bn_b") - g(p + "_bn_m") * scale
        pwm = g(p + "_pw")[:, :, 0, 0]
        const_row = pwm @ biasc
        w[p + "pwT"] = np.ascontiguousarray(
            np.concatenate([pwm.T, const_row[None, :]], 0)).astype(BF16NP)
        w[p + "dw"] = np.ascontiguousarray(dww)
    dq = w["qdw"]                                     # [C, 9] fused dw weights
    qdA = np.zeros((128, 9, 128), np.float32)
    qdA[np.arange(128), :, np.arange(128)] = dq[0:128]
    qdB = np.zeros((64, 9, 64), np.float32)
    qdB[np.arange(64), :, np.arange(64)] = dq[128:C]
    w["qdiagA"] = np.ascontiguousarray(qdA.reshape(128, 9 * 128)).astype(BF16NP)
    w["qdiagB"] = np.ascontiguousarray(qdB.reshape(64, 9 * 64)).astype(BF16NP)
    owt = g("o_w")[:, :, 0, 0].T                      # [INNER, C]
    w["owt4"] = np.ascontiguousarray(
        owt.reshape(4, 128, C).transpose(1, 0, 2).reshape(128, 4 * C)
    ).astype(BF16NP)
    w["ones128x64"] = np.ones((128, HID), BF16NP)
    w["onesq"] = np.ones((1, BPC, Lq), BF16NP)
    w["oneskv"] = np.ones((1, BPC, Lkv), BF16NP)
    w["ob"] = np.ascontiguousarray(g("o_b")[:, None])
    return w


def kernel(**inputs):
    global _NC, LAST_RESULT
    if _NC is None:
        _NC = _build()
    w = _prep_weights(inputs)
    x = np.ascontiguousarray(
        np.asarray(inputs["x"], np.float32).reshape(B, C, Lq)).astype(BF16NP)
    in_maps = []
    for c in range(NCORES):
        m = {"x": np.ascontiguousarray(x[c * BPC:(c + 1) * BPC])}
        m.update(w)
        in_maps.append(m)
    res = run_bass_kernel_spmd(_NC, in_maps, list(range(NCORES)))
    LAST_RESULT = res
    out = np.concatenate([r["out"] for r in res.results], 0)
    return np.ascontiguousarray(out.reshape(B, C, S, S).astype(np.float32))
